# revision 26
# baseline (speedup 1.0000x reference)
"""TRN2 Bass kernel for a 3-layer GAT (4 heads) + inner-product decoder.

Strategy (8 NeuronCores, SPMD):
- Host packs the 10000 graph nodes into 8 cores x CPC chunks x 128 slots,
  degree-balanced (LPT), defining a node permutation. Edges are grouped by the
  chunk of their destination and padded to 128-edge tiles.
- Per layer: row-sharded fused GEMM z|el|er = h @ [W | W@Al | W@Ar] (fp16),
  AllGather of the fp16 z_ext table, then per-chunk dma_gather of source-node
  rows. Edge softmax uses shift-invariance (no segment max) and
  divide-after-aggregate:  out[d] = (sum_e exp(e) * z[src_e]) / sum_e exp(e).
  Segment sums and er-expansion are one-hot selection matmuls on the PE,
  accumulating in PSUM per 128-destination chunk.
- Decoder: logits AllGather + row-sharded fp16 GEMM -> fp32 adj (50MB/core).
"""
import os
import numpy as np

N_NODES = 10000
H = 4
NCORES = 8
CAP = 128

_PROG_CACHE = {}


# ----------------------------------------------------------------------------
# host-side graph packing
# ----------------------------------------------------------------------------
def _pack_graph(dst, n_nodes, cpc):
    """Degree-balanced packing of nodes into NCORES*cpc bins of <=CAP nodes."""
    import heapq
    deg = np.bincount(dst, minlength=n_nodes)
    n_bins = NCORES * cpc
    order = np.argsort(-deg, kind="stable")
    bin_nodes = [[] for _ in range(n_bins)]
    heap = [(0, 0, b) for b in range(n_bins)]
    heapq.heapify(heap)
    for g in order:
        spill = []
        while True:
            load, cnt, b = heapq.heappop(heap)
            if cnt < CAP:
                bin_nodes[b].append(g)
                heapq.heappush(heap, (load + int(deg[g]), cnt + 1, b))
                break
            spill.append((load, cnt, b))
        for s in spill:
            heapq.heappush(heap, s)
    loads = np.array([int(deg[bin_nodes[b]].sum()) for b in range(n_bins)])
    rank = np.argsort(-loads, kind="stable")
    # bin ranked r -> core r%NCORES, chunk r//NCORES (equalizes chunk tile
    # counts across cores, which must be shared in the SPMD program)
    perm = np.full(n_bins * CAP, -1, np.int64)   # table row -> node (-1 = pad)
    slot_of = np.full(n_nodes, -1, np.int64)     # node -> table row
    for r, b in enumerate(rank):
        core, chunk = r % NCORES, r // NCORES
        base = (core * cpc + chunk) * CAP
        nodes = bin_nodes[b]
        perm[base:base + len(nodes)] = nodes
        slot_of[nodes] = base + np.arange(len(nodes))
    return perm, slot_of


def _prep_host(inputs, cpc):
    """All host-side preprocessing. Returns (in_maps, perm, T_list, meta)."""
    src = np.asarray(inputs["edge_src"]).astype(np.int64)
    dst = np.asarray(inputs["edge_dst"]).astype(np.int64)
    x = np.asarray(inputs["inputs"]).astype(np.float32)
    n_nodes = x.shape[0]

    perm, slot_of = _pack_graph(dst, n_nodes, cpc)

    # table rows: core shards of (cpc*CAP + 1) rows — last row of each shard
    # is a dummy (zero z, el = -30000) so padded edges contribute nothing.
    shard_rows = cpc * CAP + 1
    row_of_slot = (slot_of // (cpc * CAP)) * shard_rows + slot_of % (cpc * CAP)
    dummy = cpc * CAP                           # core 0's dummy row
    src_row = row_of_slot[src]
    dst_row = slot_of[dst]
    ebin = dst_row // CAP                       # bin = core*cpc + chunk
    order = np.argsort(ebin, kind="stable")
    src_row_s, dst_row_s, ebin_s = src_row[order], dst_row[order], ebin[order]
    counts = np.bincount(ebin_s, minlength=NCORES * cpc)
    starts = np.concatenate([[0], np.cumsum(counts)])

    T_list = []
    for k in range(cpc):
        mx = max(int(counts[c * cpc + k]) for c in range(NCORES))
        T_list.append(max(1, (mx + 127) // 128))
    TT = int(sum(T_list))
    offs = np.concatenate([[0], np.cumsum(T_list)]).astype(np.int64)

    # per-core per-edge-slot arrays (slots = sum_k T_k*128)
    nslots = TT * 128
    gsrc = np.full((NCORES, nslots), dummy, np.int64)
    gdst = np.zeros((NCORES, nslots), np.int64)  # local dst id 0..127
    for c in range(NCORES):
        for k in range(cpc):
            b = c * cpc + k
            s, e = starts[b], starts[b + 1]
            n = e - s
            base = offs[k] * 128
            gsrc[c, base:base + n] = src_row_s[s:e]
            gdst[c, base:base + n] = dst_row_s[s:e] % CAP

    # dma_gather index layout: int16, idx j at [j%16 + 16r, j//16] (replicated)
    gidx = np.zeros((NCORES, 128, nslots // 16), np.int16)
    j = np.arange(nslots)
    for c in range(NCORES):
        blk = np.zeros((16, nslots // 16), np.int16)
        blk[j % 16, j // 16] = gsrc[c].astype(np.int16)
        gidx[c] = np.tile(blk, (8, 1))
    # dstid layout: [128, TT] fp32, edge j at [j%128, j//128]
    dstid = np.zeros((NCORES, 128, TT), np.float32)
    for c in range(NCORES):
        dstid[c][j % 128, j // 128] = gdst[c].astype(np.float32)

    # local node features, table-row-ordered, p-major [128, cpc*256]
    F0 = x.shape[1]
    xloc = np.zeros((NCORES, 128, cpc * F0), np.float16)
    for c in range(NCORES):
        xt = np.zeros((cpc * CAP, F0), np.float32)
        rows = perm[c * cpc * CAP:(c + 1) * cpc * CAP]
        v = rows >= 0
        xt[v] = x[rows[v]]
        xloc[c] = xt.reshape(cpc, 128, F0).transpose(1, 0, 2).reshape(128, -1)

    # fused weights [W | W@Al | W@Ar] -> [128, K*C] fp16
    def fuse(W, al, ar):
        Fin, HD = W.shape
        D = HD // H
        Al = np.zeros((HD, H), np.float32)
        Ar = np.zeros((HD, H), np.float32)
        for hh in range(H):
            Al[hh * D:(hh + 1) * D, hh] = al[hh]
            Ar[hh * D:(hh + 1) * D, hh] = ar[hh]
        Wf = np.concatenate([W, W @ Al, W @ Ar], axis=1).astype(np.float16)
        K = Fin // 128
        C = Wf.shape[1]
        return Wf.reshape(K, 128, C).transpose(1, 0, 2).reshape(128, K * C)

    g = lambda k: np.asarray(inputs[k]).astype(np.float32)
    w0 = fuse(g("W0"), g("al0"), g("ar0"))
    w1 = fuse(g("W1"), g("al1"), g("ar1"))
    w2 = fuse(g("W2"), g("al2"), g("ar2"))
    bias01 = np.concatenate([
        np.broadcast_to(g("b0").reshape(-1), (128, 512)),
        np.broadcast_to(g("b1").reshape(-1), (128, 512))], axis=1
    ).astype(np.float16)
    bias2 = np.broadcast_to(g("b2").mean(axis=0), (128, 64)).astype(np.float32)

    ident = np.eye(128, dtype=np.float16)
    iotaf = np.broadcast_to(np.arange(128, dtype=np.float32), (128, 128))
    in_maps = [{
        "xloc": np.ascontiguousarray(xloc[c]),
        "w0": w0, "w1": w1, "w2": w2,
        "bias01": bias01, "bias2": np.ascontiguousarray(bias2),
        "gidx": np.ascontiguousarray(gidx[c]),
        "dstid": np.ascontiguousarray(dstid[c]),
        "ident": ident, "iotaf": np.ascontiguousarray(iotaf),
    } for c in range(NCORES)]
    return in_maps, perm, T_list


# ----------------------------------------------------------------------------
# device program
# ----------------------------------------------------------------------------
def _build_program(T_list, cpc):
    from contextlib import ExitStack
    import concourse.bass as bass
    import concourse.tile as tile
    from concourse import bacc, mybir

    f16, f32 = mybir.dt.float16, mybir.dt.float32
    i16, i32 = mybir.dt.int16, mybir.dt.int32
    AF = mybir.ActivationFunctionType
    OP = mybir.AluOpType

    cfg = [  # per layer: Fin, zc, ext
        (256, 512, 640),
        (512, 512, 640),
        (512, 256, 384),
    ]
    TT = int(sum(T_list))
    Tmax = max(T_list)
    offs = np.concatenate([[0], np.cumsum(T_list)]).astype(np.int64)
    nrows = NCORES * cpc * CAP
    shard_rows = cpc * CAP + 1
    tbl_rows = NCORES * shard_rows + 8

    stage = int(os.environ.get("KSTAGE", "9"))
    nc = bacc.Bacc("TRN2", target_bir_lowering=False, debug=False,
                   enable_asserts=False, num_devices=NCORES)
    xloc_d = nc.dram_tensor("xloc", [128, cpc * 256], f16, kind="ExternalInput").ap()
    w_d = [nc.dram_tensor(f"w{l}", [128, (cfg[l][0] // 128) * (cfg[l][1] + 8)],
                          f16, kind="ExternalInput").ap() for l in range(3)]
    bias01_d = nc.dram_tensor("bias01", [128, 1024], f16, kind="ExternalInput").ap()
    bias2_d = nc.dram_tensor("bias2", [128, 64], f32, kind="ExternalInput").ap()
    gidx_d = nc.dram_tensor("gidx", [128, TT * 8], i16, kind="ExternalInput").ap()
    dstid_d = nc.dram_tensor("dstid", [128, TT], f32, kind="ExternalInput").ap()
    ident_d = nc.dram_tensor("ident", [128, 128], f16, kind="ExternalInput").ap()
    iotaf_d = nc.dram_tensor("iotaf", [128, 128], f32, kind="ExternalInput").ap()
    adj_d = nc.dram_tensor("adj", [cpc * CAP, nrows], f32, kind="ExternalOutput").ap()
    logits_d = nc.dram_tensor("logits", [cpc * CAP, 64], f32, kind="ExternalOutput").ap()

    with tile.TileContext(nc) as tc, ExitStack() as ctx:
        const = ctx.enter_context(tc.tile_pool(name="const", bufs=1))
        hp = ctx.enter_context(tc.tile_pool(name="hp", bufs=2))
        gp = ctx.enter_context(tc.tile_pool(name="gp", bufs=2))
        wk = ctx.enter_context(tc.tile_pool(name="wk", bufs=2))
        sm = ctx.enter_context(tc.tile_pool(name="sm", bufs=3))
        ps = ctx.enter_context(tc.tile_pool(name="ps", bufs=2, space="PSUM"))
        dram = ctx.enter_context(tc.tile_pool(name="dram", bufs=1, space="DRAM"))

        # ---- constants (host-supplied; keeps gpsimd mlp-library-only) ----
        id16 = const.tile([128, 128], f16)
        nc.sync.dma_start(id16[:], ident_d[:])
        iota_f = const.tile([128, 128], f32)
        nc.sync.dma_start(iota_f[:], iotaf_d[:])
        dummy = const.tile([1, 640], f16)
        nc.vector.memset(dummy[:], 0.0)
        nc.vector.memset(dummy[0:1, 256:260], -30000.0)
        nc.vector.memset(dummy[0:1, 512:516], -30000.0)

        xloc = const.tile([128, cpc * 256], f16)
        nc.sync.dma_start(xloc[:], xloc_d[:])
        wsb = []
        for l in range(3):
            wt = const.tile([128, w_d[l].shape[1]], f16, name=f"wsb{l}")
            nc.sync.dma_start(wt[:], w_d[l][:])
            wsb.append(wt)
        bias01 = const.tile([128, 1024], f16)
        nc.sync.dma_start(bias01[:], bias01_d[:])
        bias2 = const.tile([128, 64], f32)
        nc.sync.dma_start(bias2[:], bias2_d[:])
        gidx = const.tile([128, TT * 8], i16)
        nc.sync.dma_start(gidx[:], gidx_d[:])
        dstid = const.tile([128, TT], f32)
        nc.sync.dma_start(dstid[:], dstid_d[:])

        logits32 = const.tile([128, cpc * 64], f32)
        ltT = const.tile([64, cpc * 128], f16)

        reps = int(os.environ.get("KREPS", "1"))
        for rep in range(reps):
          h_cur = xloc  # [128, cpc*Fin] fp16
          for l in range(3):
            if stage < 2 or (l > 0 and stage < 6):
                break
            Fin, zc, ext = cfg[l]
            K = Fin // 128
            C = zc + 8
            last = l == 2

            zshard = dram.tile([shard_rows, ext], f16, name=f"zshard{l}_{rep}")
            ztab = dram.tile([tbl_rows, ext], f16, addr_space="Shared",
                             name=f"ztab{l}_{rep}")
            nc.sync.dma_start(zshard[cpc * CAP:cpc * CAP + 1, :],
                              dummy[0:1, 0:ext])
            er_sb = wk.tile([128, cpc * 4], f16, name=f"er{l}", tag="er")
            if not last:
                h_next = hp.tile([128, cpc * zc], f16, name=f"h{l}", tag="h")

            # ---- GEMM phase: z|el|er for local nodes ----
            for c in range(cpc):
                hT = wk.tile([128, K * 128], f16, name="hT", tag="hT")
                for k in range(K):
                    tr = ps.tile([128, 128], f16, name="tr", tag="tr")
                    nc.tensor.transpose(
                        out=tr[:], in_=h_cur[:, c * Fin + k * 128: c * Fin + (k + 1) * 128],
                        identity=id16[:])
                    eng = nc.scalar if k % 2 else nc.vector
                    if eng is nc.scalar:
                        nc.scalar.activation(hT[:, k * 128:(k + 1) * 128], tr[:], AF.Copy)
                    else:
                        nc.vector.tensor_copy(hT[:, k * 128:(k + 1) * 128], tr[:])
                z_ps = ps.tile([128, 512], f32, name="z_ps", tag="big")
                s_ps = ps.tile([128, 8], f32, name="s_ps", tag="acc8")
                for k in range(K):
                    nc.tensor.matmul(out=z_ps[:, 0:zc],
                                     lhsT=hT[:, k * 128:(k + 1) * 128],
                                     rhs=wsb[l][:, k * C:k * C + zc],
                                     start=(k == 0), stop=(k == K - 1))
                for k in range(K):
                    nc.tensor.matmul(out=s_ps[:],
                                     lhsT=hT[:, k * 128:(k + 1) * 128],
                                     rhs=wsb[l][:, k * C + zc:(k + 1) * C],
                                     start=(k == 0), stop=(k == K - 1))
                zext = wk.tile([128, ext], f16, name="zext", tag="zext")
                nc.vector.tensor_copy(zext[:, 0:zc], z_ps[:, 0:zc])
                nc.scalar.activation(zext[:, zc:zc + 8], s_ps[:], AF.Copy)
                nc.vector.tensor_copy(er_sb[:, c * 4:(c + 1) * 4], s_ps[:, 4:8])
                nc.sync.dma_start(zshard[c * CAP:(c + 1) * CAP, :], zext[:])

            if stage < 3:
                break
            nc.gpsimd.collective_compute(
                "AllGather", mybir.AluOpType.bypass,
                replica_groups=[list(range(NCORES))],
                ins=[zshard[:]], outs=[ztab[0:NCORES * shard_rows, :]])

            if stage < 4:
                break
            # ---- edge phase ----
            for c in range(cpc):
                T = T_list[c]
                off = int(offs[c])
                g_sb = gp.tile([128, Tmax, ext], f16, name="g_sb", tag="g")
                idxc = wk.tile([128, Tmax * 8], i16, name="idxc", tag="idxc")
                nc.vector.tensor_copy(idxc[:, 0:T * 8], gidx[:, off * 8:(off + T) * 8])
                nc.gpsimd.dma_gather(
                    g_sb[:, 0:T, 0:ext], ztab[0:NCORES * shard_rows, :],
                    idxc[:, 0:T * 8], T * 128, T * 128, ext,
                    single_packet=False)
                if stage < 5:
                    continue
                un_ps = ps.tile([128, 512], f32, name="un_ps", tag="big")
                den_ps = None if last else ps.tile([128, 8], f32, name="den_ps",
                                                   tag="acc8")
                for t in range(T):
                    S = sm.tile([128, 128], f16, name="S", tag="S")
                    nc.vector.tensor_tensor(
                        out=S[:], in0=dstid[:, off + t:off + t + 1].to_broadcast([128, 128]),
                        in1=iota_f[:], op=OP.is_equal)
                    st_ps = ps.tile([128, 128], f16, name="st_ps", tag="tr")
                    nc.tensor.transpose(out=st_ps[:], in_=S[:], identity=id16[:])
                    st = sm.tile([128, 128], f16, name="st", tag="st")
                    nc.scalar.activation(st[:], st_ps[:], AF.Copy)
                    er_ps = ps.tile([128, 8], f32, name="er_ps", tag="er8")
                    nc.tensor.matmul(out=er_ps[:, 0:4], lhsT=st[:],
                                     rhs=er_sb[:, c * 4:(c + 1) * 4],
                                     start=True, stop=True)
                    e32 = sm.tile([128, 4], f32, name="e32", tag="e32")
                    nc.vector.tensor_tensor(out=e32[:], in0=g_sb[:, t, zc:zc + 4],
                                            in1=er_ps[:, 0:4], op=OP.add)
                    pr32 = sm.tile([128, 4], f32, name="pr32", tag="pr32")
                    nc.scalar.activation(pr32[:], e32[:], AF.Prelu, alpha=0.2)
                    p32 = sm.tile([128, 4], f32, name="p32", tag="p32")
                    nc.scalar.activation(p32[:], pr32[:], AF.Exp)
                    msg = sm.tile([128, zc + 8], f16, name="msg", tag="msg")
                    nc.vector.tensor_copy(msg[:, zc:zc + 4], p32[:])
                    # msg = p16 * z_src, heads split across DVE / ACT / GPSIMD
                    D = zc // H
                    nc.vector.tensor_tensor(
                        out=msg[:, 0:2 * D].rearrange("p (h d) -> p h d", h=2),
                        in0=g_sb[:, t, 0:2 * D].rearrange("p (h d) -> p h d", h=2),
                        in1=msg[:, zc:zc + 2].to_broadcast([128, 2, D]),
                        op=OP.mult)
                    nc.scalar.activation(msg[:, 2 * D:3 * D], g_sb[:, t, 2 * D:3 * D],
                                         AF.Copy, scale=p32[:, 2:3])
                    nc.scalar.activation(msg[:, 3 * D:4 * D], g_sb[:, t, 3 * D:4 * D],
                                         AF.Copy, scale=p32[:, 3:4])
                    if last:
                        nc.tensor.matmul(out=un_ps[:, 0:zc + 4], lhsT=S[:],
                                         rhs=msg[:, 0:zc + 4],
                                         start=(t == 0), stop=(t == T - 1))
                    else:
                        nc.tensor.matmul(out=un_ps[:, 0:zc], lhsT=S[:],
                                         rhs=msg[:, 0:zc],
                                         start=(t == 0), stop=(t == T - 1))
                        nc.tensor.matmul(out=den_ps[:, 0:4], lhsT=S[:],
                                         rhs=msg[:, zc:zc + 4],
                                         start=(t == 0), stop=(t == T - 1))

                # ---- chunk epilogue ----
                den32 = sm.tile([128, 4], f32, name="den32", tag="den32")
                if last:
                    nc.vector.tensor_scalar_add(den32[:], un_ps[:, zc:zc + 4], 1e-30)
                else:
                    nc.vector.tensor_scalar_add(den32[:], den_ps[:, 0:4], 1e-30)
                recip = sm.tile([128, 4], f32, name="recip", tag="recip")
                nc.vector.reciprocal(recip[:], den32[:])
                if last:
                    nc.vector.tensor_scalar_mul(recip[:], recip[:], 0.25)
                D = zc // H
                outn = sm.tile([128, 512], f32, name="outn", tag="outn")
                nc.vector.tensor_tensor(
                    out=outn[:, 0:zc].rearrange("p (h d) -> p h d", h=H),
                    in0=un_ps[:, 0:zc].rearrange("p (h d) -> p h d", h=H),
                    in1=recip[:].to_broadcast([128, H, D]),
                    op=OP.mult)
                if last:
                    lg = logits32[:, c * 64:(c + 1) * 64]
                    nc.vector.tensor_add(outn[:, 0:64], outn[:, 0:64], outn[:, 64:128])
                    nc.vector.tensor_add(outn[:, 128:192], outn[:, 128:192],
                                         outn[:, 192:256])
                    nc.vector.tensor_add(outn[:, 0:64], outn[:, 0:64], outn[:, 128:192])
                    nc.vector.tensor_add(lg[:], outn[:, 0:64], bias2[:])
                    nc.sync.dma_start(logits_d[c * CAP:(c + 1) * CAP, :], lg[:])
                    l16 = sm.tile([128, 64], f16, name="l16", tag="l16")
                    nc.vector.tensor_copy(l16[:], lg[:])
                    ltr = ps.tile([128, 128], f16, name="ltr", tag="tr")
                    nc.tensor.transpose(out=ltr[0:64, :], in_=l16[:], identity=id16[:])
                    nc.scalar.activation(ltT[:, c * 128:(c + 1) * 128], ltr[0:64, :],
                                         AF.Copy)
                else:
                    badd = sm.tile([128, 512], f32, name="badd", tag="badd")
                    nc.vector.tensor_tensor(out=badd[:, 0:zc], in0=outn[:, 0:zc],
                                            in1=bias01[:, l * 512:l * 512 + zc],
                                            op=OP.add)
                    nc.scalar.activation(h_next[:, c * zc:(c + 1) * zc],
                                         badd[:, 0:zc], AF.Relu)
            if not last:
                h_cur = h_next

          # ---- decoder ----
          if stage >= 7:
            lt_shard = dram.tile([64, cpc * 128], f16, name=f"lt_shard_{rep}")
            lt_all = dram.tile([NCORES * 64, cpc * 128], f16, addr_space="Shared",
                               name=f"lt_all_{rep}")
            nc.sync.dma_start(lt_shard[:], ltT[:])
            nc.gpsimd.collective_compute(
                "AllGather", mybir.AluOpType.bypass,
                replica_groups=[list(range(NCORES))],
                ins=[lt_shard[:]], outs=[lt_all[:]])
            ltfull = const.tile([64, nrows], f16)
            for g in range(NCORES):
                nc.sync.dma_start(ltfull[0:64, g * cpc * 128:(g + 1) * cpc * 128],
                                  lt_all[g * 64:(g + 1) * 64, :])

            nblocks = nrows // 512
            for c in range(cpc):
                for nb2 in range(nblocks // 2):
                    adj_sb = wk.tile([128, 1024], f32, name="adj_sb", tag="adj")
                    for half in range(2):
                        nb = nb2 * 2 + half
                        adj_ps = ps.tile([128, 512], f32, name="adj_ps", tag="big")
                        nc.tensor.matmul(out=adj_ps[:],
                                         lhsT=ltT[:, c * 128:(c + 1) * 128],
                                         rhs=ltfull[:, nb * 512:(nb + 1) * 512],
                                         start=True, stop=True)
                        if half:
                            nc.scalar.activation(adj_sb[:, half * 512:(half + 1) * 512],
                                                 adj_ps[:], AF.Copy)
                        else:
                            nc.vector.tensor_copy(adj_sb[:, half * 512:(half + 1) * 512],
                                                  adj_ps[:])
                    nc.sync.dma_start(
                        adj_d[c * CAP:(c + 1) * CAP, nb2 * 1024:(nb2 + 1) * 1024],
                        adj_sb[:])

    nc.compile()
    return nc


# ----------------------------------------------------------------------------
# entry point
# ----------------------------------------------------------------------------
def kernel(**inputs):
    from concourse.bass_utils import run_bass_kernel_spmd

    n_nodes = np.asarray(inputs["inputs"]).shape[0]
    cpc = max(1, (n_nodes + NCORES * CAP - 1) // (NCORES * CAP))
    in_maps, perm, T_list = _prep_host(inputs, cpc)

    key = (tuple(T_list), cpc, os.environ.get("KSTAGE", "9"),
           os.environ.get("KREPS", "1"))
    if key not in _PROG_CACHE:
        _PROG_CACHE[key] = _build_program(T_list, cpc)
    nc = _PROG_CACHE[key]

    trace = bool(int(os.environ.get("KERNEL_TRACE", "0")))
    res = run_bass_kernel_spmd(nc, in_maps, core_ids=list(range(NCORES)),
                               trace=trace)
    if trace:
        kernel.last_exec_time_ns = res.exec_time_ns
        kernel.last_results = res

    nrows = NCORES * cpc * CAP
    valid = perm >= 0
    cols = perm[valid]                       # table col -> node, for valid cols
    adj = np.zeros((n_nodes, n_nodes), np.float32)
    logits = np.zeros((n_nodes, 64), np.float32)
    for c in range(NCORES):
        r = res.results[c]
        pv = perm[c * cpc * CAP:(c + 1) * cpc * CAP]
        rv = pv >= 0
        logits[pv[rv]] = r["logits"][rv]
        adj[np.ix_(pv[rv], cols)] = r["adj"][np.ix_(rv, valid)]
    return adj, logits


# revision 29
# speedup vs baseline: 1.0137x; 1.0137x over previous
"""TRN2 Bass kernel for a 3-layer GAT (4 heads) + inner-product decoder.

Strategy (8 NeuronCores, SPMD):
- Host packs the 10000 graph nodes into 8 cores x CPC chunks x 128 slots,
  degree-balanced (LPT), defining a node permutation. Edges are grouped by the
  chunk of their destination and padded to 128-edge tiles.
- Per layer: row-sharded fused GEMM z|el|er = h @ [W | W@Al | W@Ar] (fp16),
  AllGather of the fp16 z_ext table, then per-chunk dma_gather of source-node
  rows. Edge softmax uses shift-invariance (no segment max) and
  divide-after-aggregate:  out[d] = (sum_e exp(e) * z[src_e]) / sum_e exp(e).
  Segment sums and er-expansion are one-hot selection matmuls on the PE,
  accumulating in PSUM per 128-destination chunk.
- Decoder: logits AllGather + row-sharded fp16 GEMM -> fp32 adj (50MB/core).
"""
import os
import numpy as np

N_NODES = 10000
H = 4
NCORES = 8
CAP = 128

_PROG_CACHE = {}


# ----------------------------------------------------------------------------
# host-side graph packing
# ----------------------------------------------------------------------------
def _pack_graph(dst, n_nodes, cpc):
    """Degree-balanced packing of nodes into NCORES*cpc bins of <=CAP nodes."""
    import heapq
    deg = np.bincount(dst, minlength=n_nodes)
    n_bins = NCORES * cpc
    order = np.argsort(-deg, kind="stable")
    bin_nodes = [[] for _ in range(n_bins)]
    heap = [(0, 0, b) for b in range(n_bins)]
    heapq.heapify(heap)
    for g in order:
        spill = []
        while True:
            load, cnt, b = heapq.heappop(heap)
            if cnt < CAP:
                bin_nodes[b].append(g)
                heapq.heappush(heap, (load + int(deg[g]), cnt + 1, b))
                break
            spill.append((load, cnt, b))
        for s in spill:
            heapq.heappush(heap, s)
    loads = np.array([int(deg[bin_nodes[b]].sum()) for b in range(n_bins)])
    rank = np.argsort(-loads, kind="stable")
    # bin ranked r -> core r%NCORES, chunk r//NCORES (equalizes chunk tile
    # counts across cores, which must be shared in the SPMD program)
    perm = np.full(n_bins * CAP, -1, np.int64)   # table row -> node (-1 = pad)
    slot_of = np.full(n_nodes, -1, np.int64)     # node -> table row
    for r, b in enumerate(rank):
        core, chunk = r % NCORES, r // NCORES
        base = (core * cpc + chunk) * CAP
        nodes = bin_nodes[b]
        perm[base:base + len(nodes)] = nodes
        slot_of[nodes] = base + np.arange(len(nodes))
    return perm, slot_of


def _prep_host(inputs, cpc):
    """All host-side preprocessing. Returns (in_maps, perm, T_list, meta)."""
    src = np.asarray(inputs["edge_src"]).astype(np.int64)
    dst = np.asarray(inputs["edge_dst"]).astype(np.int64)
    x = np.asarray(inputs["inputs"]).astype(np.float32)
    n_nodes = x.shape[0]

    perm, slot_of = _pack_graph(dst, n_nodes, cpc)

    # table rows: core shards of (cpc*CAP + 1) rows — last row of each shard
    # is a dummy (zero z, el = -30000) so padded edges contribute nothing.
    shard_rows = cpc * CAP + 1
    row_of_slot = (slot_of // (cpc * CAP)) * shard_rows + slot_of % (cpc * CAP)
    dummy = cpc * CAP                           # core 0's dummy row
    src_row = row_of_slot[src]
    dst_row = slot_of[dst]
    ebin = dst_row // CAP                       # bin = core*cpc + chunk
    order = np.argsort(ebin, kind="stable")
    src_row_s, dst_row_s, ebin_s = src_row[order], dst_row[order], ebin[order]
    counts = np.bincount(ebin_s, minlength=NCORES * cpc)
    starts = np.concatenate([[0], np.cumsum(counts)])

    T_list = []
    for k in range(cpc):
        mx = max(int(counts[c * cpc + k]) for c in range(NCORES))
        T_list.append(max(1, (mx + 127) // 128))
    TT = int(sum(T_list))
    offs = np.concatenate([[0], np.cumsum(T_list)]).astype(np.int64)

    # per-core per-edge-slot arrays (slots = sum_k T_k*128)
    nslots = TT * 128
    gsrc = np.full((NCORES, nslots), dummy, np.int64)
    gdst = np.zeros((NCORES, nslots), np.int64)  # local dst id 0..127
    for c in range(NCORES):
        for k in range(cpc):
            b = c * cpc + k
            s, e = starts[b], starts[b + 1]
            n = e - s
            base = offs[k] * 128
            gsrc[c, base:base + n] = src_row_s[s:e]
            gdst[c, base:base + n] = dst_row_s[s:e] % CAP

    # dma_gather index layout: int16, idx j at [j%16 + 16r, j//16] (replicated)
    gidx = np.zeros((NCORES, 128, nslots // 16), np.int16)
    j = np.arange(nslots)
    for c in range(NCORES):
        blk = np.zeros((16, nslots // 16), np.int16)
        blk[j % 16, j // 16] = gsrc[c].astype(np.int16)
        gidx[c] = np.tile(blk, (8, 1))
    # dstid layout: [128, TT] fp32, edge j at [j%128, j//128]
    dstid = np.zeros((NCORES, 128, TT), np.float32)
    for c in range(NCORES):
        dstid[c][j % 128, j // 128] = gdst[c].astype(np.float32)

    # local node features, table-row-ordered, p-major [128, cpc*256]
    F0 = x.shape[1]
    xloc = np.zeros((NCORES, 128, cpc * F0), np.float16)
    for c in range(NCORES):
        xt = np.zeros((cpc * CAP, F0), np.float32)
        rows = perm[c * cpc * CAP:(c + 1) * cpc * CAP]
        v = rows >= 0
        xt[v] = x[rows[v]]
        xloc[c] = xt.reshape(cpc, 128, F0).transpose(1, 0, 2).reshape(128, -1)

    # fused weights [W | W@Al | W@Ar] -> [128, K*C] fp16
    def fuse(W, al, ar):
        Fin, HD = W.shape
        D = HD // H
        Al = np.zeros((HD, H), np.float32)
        Ar = np.zeros((HD, H), np.float32)
        for hh in range(H):
            Al[hh * D:(hh + 1) * D, hh] = al[hh]
            Ar[hh * D:(hh + 1) * D, hh] = ar[hh]
        Wf = np.concatenate([W, W @ Al, W @ Ar], axis=1).astype(np.float16)
        K = Fin // 128
        C = Wf.shape[1]
        return Wf.reshape(K, 128, C).transpose(1, 0, 2).reshape(128, K * C)

    g = lambda k: np.asarray(inputs[k]).astype(np.float32)
    w0 = fuse(g("W0"), g("al0"), g("ar0"))
    w1 = fuse(g("W1"), g("al1"), g("ar1"))
    w2 = fuse(g("W2"), g("al2"), g("ar2"))
    bias01 = np.concatenate([
        np.broadcast_to(g("b0").reshape(-1), (128, 512)),
        np.broadcast_to(g("b1").reshape(-1), (128, 512))], axis=1
    ).astype(np.float16)
    bias2 = np.broadcast_to(g("b2").mean(axis=0), (128, 64)).astype(np.float32)

    ident = np.eye(128, dtype=np.float16)
    iotaf = np.broadcast_to(np.arange(128, dtype=np.float32), (128, 128))
    in_maps = [{
        "xloc": np.ascontiguousarray(xloc[c]),
        "w0": w0, "w1": w1, "w2": w2,
        "bias01": bias01, "bias2": np.ascontiguousarray(bias2),
        "gidx": np.ascontiguousarray(gidx[c]),
        "dstid": np.ascontiguousarray(dstid[c]),
        "ident": ident, "iotaf": np.ascontiguousarray(iotaf),
    } for c in range(NCORES)]
    return in_maps, perm, T_list


# ----------------------------------------------------------------------------
# device program
# ----------------------------------------------------------------------------
def _build_program(T_list, cpc):
    from contextlib import ExitStack
    import concourse.bass as bass
    import concourse.tile as tile
    from concourse import bacc, mybir

    f16, f32 = mybir.dt.float16, mybir.dt.float32
    i16, i32 = mybir.dt.int16, mybir.dt.int32
    AF = mybir.ActivationFunctionType
    OP = mybir.AluOpType

    cfg = [  # per layer: Fin, zc, ext
        (256, 512, 640),
        (512, 512, 640),
        (512, 256, 384),
    ]
    TT = int(sum(T_list))
    Tmax = max(T_list)
    offs = np.concatenate([[0], np.cumsum(T_list)]).astype(np.int64)
    nrows = NCORES * cpc * CAP
    shard_rows = cpc * CAP + 1
    tbl_rows = NCORES * shard_rows + 8

    stage = int(os.environ.get("KSTAGE", "9"))
    nc = bacc.Bacc("TRN2", target_bir_lowering=False, debug=False,
                   enable_asserts=False, num_devices=NCORES)
    xloc_d = nc.dram_tensor("xloc", [128, cpc * 256], f16, kind="ExternalInput").ap()
    w_d = [nc.dram_tensor(f"w{l}", [128, (cfg[l][0] // 128) * (cfg[l][1] + 8)],
                          f16, kind="ExternalInput").ap() for l in range(3)]
    bias01_d = nc.dram_tensor("bias01", [128, 1024], f16, kind="ExternalInput").ap()
    bias2_d = nc.dram_tensor("bias2", [128, 64], f32, kind="ExternalInput").ap()
    gidx_d = nc.dram_tensor("gidx", [128, TT * 8], i16, kind="ExternalInput").ap()
    dstid_d = nc.dram_tensor("dstid", [128, TT], f32, kind="ExternalInput").ap()
    ident_d = nc.dram_tensor("ident", [128, 128], f16, kind="ExternalInput").ap()
    iotaf_d = nc.dram_tensor("iotaf", [128, 128], f32, kind="ExternalInput").ap()
    adj_d = nc.dram_tensor("adj", [cpc * CAP, nrows], f32, kind="ExternalOutput").ap()
    logits_d = nc.dram_tensor("logits", [cpc * CAP, 64], f32, kind="ExternalOutput").ap()

    with tile.TileContext(nc) as tc, ExitStack() as ctx:
        const = ctx.enter_context(tc.tile_pool(name="const", bufs=1))
        hp = ctx.enter_context(tc.tile_pool(name="hp", bufs=2))
        gp = ctx.enter_context(tc.tile_pool(name="gp", bufs=2))
        wk = ctx.enter_context(tc.tile_pool(name="wk", bufs=2))
        sm = ctx.enter_context(tc.tile_pool(name="sm", bufs=3))
        ps = ctx.enter_context(tc.tile_pool(name="ps", bufs=2, space="PSUM"))
        dram = ctx.enter_context(tc.tile_pool(name="dram", bufs=1, space="DRAM"))

        # ---- constants (host-supplied; keeps gpsimd mlp-library-only) ----
        id16 = const.tile([128, 128], f16)
        nc.sync.dma_start(id16[:], ident_d[:])
        iota_f = const.tile([128, 128], f32)
        nc.sync.dma_start(iota_f[:], iotaf_d[:])
        dummy = const.tile([1, 640], f16)
        nc.vector.memset(dummy[:], 0.0)
        nc.vector.memset(dummy[0:1, 256:260], -30000.0)
        nc.vector.memset(dummy[0:1, 512:516], -30000.0)

        xloc = const.tile([128, cpc * 256], f16)
        nc.sync.dma_start(xloc[:], xloc_d[:])
        wsb = []
        for l in range(3):
            wt = const.tile([128, w_d[l].shape[1]], f16, name=f"wsb{l}")
            nc.sync.dma_start(wt[:], w_d[l][:])
            wsb.append(wt)
        bias01 = const.tile([128, 1024], f16)
        nc.sync.dma_start(bias01[:], bias01_d[:])
        bias2 = const.tile([128, 64], f32)
        nc.sync.dma_start(bias2[:], bias2_d[:])
        gidx = const.tile([128, TT * 8], i16)
        nc.sync.dma_start(gidx[:], gidx_d[:])
        dstid = const.tile([128, TT], f32)
        nc.sync.dma_start(dstid[:], dstid_d[:])

        logits32 = const.tile([128, cpc * 64], f32)
        ltT = const.tile([64, cpc * 128], f16)

        reps = int(os.environ.get("KREPS", "1"))
        for rep in range(reps):
          h_cur = xloc  # [128, cpc*Fin] fp16
          for l in range(3):
            if stage < 2 or (l > 0 and stage < 6):
                break
            Fin, zc, ext = cfg[l]
            K = Fin // 128
            C = zc + 8
            last = l == 2

            zshard = dram.tile([shard_rows, ext], f16, name=f"zshard{l}_{rep}")
            ztab = dram.tile([tbl_rows, ext], f16, addr_space="Shared",
                             name=f"ztab{l}_{rep}")
            nc.sync.dma_start(zshard[cpc * CAP:cpc * CAP + 1, :],
                              dummy[0:1, 0:ext])
            er_sb = wk.tile([128, cpc * 4], f16, name=f"er{l}", tag="er")
            if not last:
                h_next = hp.tile([128, cpc * zc], f16, name=f"h{l}", tag="h")

            # ---- GEMM phase: z|el|er for local nodes ----
            for c in range(cpc):
                hT = wk.tile([128, K * 128], f16, name="hT", tag="hT")
                for k in range(K):
                    tr = ps.tile([128, 128], f16, name="tr", tag="tr")
                    nc.tensor.transpose(
                        out=tr[:], in_=h_cur[:, c * Fin + k * 128: c * Fin + (k + 1) * 128],
                        identity=id16[:])
                    eng = nc.scalar if k % 2 else nc.vector
                    if eng is nc.scalar:
                        nc.scalar.activation(hT[:, k * 128:(k + 1) * 128], tr[:], AF.Copy)
                    else:
                        nc.vector.tensor_copy(hT[:, k * 128:(k + 1) * 128], tr[:])
                z_ps = ps.tile([128, 512], f32, name="z_ps", tag="big")
                s_ps = ps.tile([128, 8], f32, name="s_ps", tag="acc8")
                for k in range(K):
                    nc.tensor.matmul(out=z_ps[:, 0:zc],
                                     lhsT=hT[:, k * 128:(k + 1) * 128],
                                     rhs=wsb[l][:, k * C:k * C + zc],
                                     start=(k == 0), stop=(k == K - 1))
                for k in range(K):
                    nc.tensor.matmul(out=s_ps[:],
                                     lhsT=hT[:, k * 128:(k + 1) * 128],
                                     rhs=wsb[l][:, k * C + zc:(k + 1) * C],
                                     start=(k == 0), stop=(k == K - 1))
                zext = wk.tile([128, ext], f16, name="zext", tag="zext")
                nc.vector.tensor_copy(zext[:, 0:zc], z_ps[:, 0:zc])
                nc.scalar.activation(zext[:, zc:zc + 8], s_ps[:], AF.Copy)
                nc.vector.tensor_copy(er_sb[:, c * 4:(c + 1) * 4], s_ps[:, 4:8])
                nc.sync.dma_start(zshard[c * CAP:(c + 1) * CAP, :], zext[:])

            if stage < 3:
                break
            nc.gpsimd.collective_compute(
                "AllGather", mybir.AluOpType.bypass,
                replica_groups=[list(range(NCORES))],
                ins=[zshard[:]], outs=[ztab[0:NCORES * shard_rows, :]])

            if stage < 4:
                break
            # ---- edge phase ----
            for c in range(cpc):
                T = T_list[c]
                off = int(offs[c])
                g_sb = gp.tile([128, Tmax, ext], f16, name="g_sb", tag="g")
                idxc = wk.tile([128, Tmax * 8], i16, name="idxc", tag="idxc")
                nc.vector.tensor_copy(idxc[:, 0:T * 8], gidx[:, off * 8:(off + T) * 8])
                nc.gpsimd.dma_gather(
                    g_sb[:, 0:T, 0:ext], ztab[0:NCORES * shard_rows, :],
                    idxc[:, 0:T * 8], T * 128, T * 128, ext,
                    single_packet=False)
                if stage < 5:
                    continue
                un_ps = ps.tile([128, 512], f32, name="un_ps", tag="big")
                den_ps = None if last else ps.tile([128, 8], f32, name="den_ps",
                                                   tag="acc8")
                for t in range(T):
                    S = sm.tile([128, 128], f16, name="S", tag="S")
                    nc.vector.tensor_tensor(
                        out=S[:], in0=dstid[:, off + t:off + t + 1].to_broadcast([128, 128]),
                        in1=iota_f[:], op=OP.is_equal)
                    st_ps = ps.tile([128, 128], f16, name="st_ps", tag="tr")
                    nc.tensor.transpose(out=st_ps[:], in_=S[:], identity=id16[:])
                    st = sm.tile([128, 128], f16, name="st", tag="st")
                    nc.scalar.activation(st[:], st_ps[:], AF.Copy)
                    er_ps = ps.tile([128, 8], f32, name="er_ps", tag="er8")
                    nc.tensor.matmul(out=er_ps[:, 0:4], lhsT=st[:],
                                     rhs=er_sb[:, c * 4:(c + 1) * 4],
                                     start=True, stop=True)
                    e32 = sm.tile([128, 4], f32, name="e32", tag="e32")
                    nc.vector.tensor_tensor(out=e32[:], in0=g_sb[:, t, zc:zc + 4],
                                            in1=er_ps[:, 0:4], op=OP.add)
                    pr32 = sm.tile([128, 4], f32, name="pr32", tag="pr32")
                    nc.scalar.activation(pr32[:], e32[:], AF.Prelu, alpha=0.2)
                    p32 = sm.tile([128, 4], f32, name="p32", tag="p32")
                    nc.scalar.activation(p32[:], pr32[:], AF.Exp)
                    msg = sm.tile([128, zc + 8], f16, name="msg", tag="msg")
                    nc.vector.tensor_copy(msg[:, zc:zc + 4], p32[:])
                    # msg = p16 * z_src, heads split across DVE / ACT
                    D = zc // H
                    nc.vector.tensor_tensor(
                        out=msg[:, 0:2 * D].rearrange("p (h d) -> p h d", h=2),
                        in0=g_sb[:, t, 0:2 * D].rearrange("p (h d) -> p h d", h=2),
                        in1=msg[:, zc:zc + 2].to_broadcast([128, 2, D]),
                        op=OP.mult)
                    nc.scalar.activation(msg[:, 2 * D:3 * D], g_sb[:, t, 2 * D:3 * D],
                                         AF.Copy, scale=p32[:, 2:3])
                    nc.scalar.activation(msg[:, 3 * D:4 * D], g_sb[:, t, 3 * D:4 * D],
                                         AF.Copy, scale=p32[:, 3:4])
                    if last:
                        nc.tensor.matmul(out=un_ps[:, 0:zc + 4], lhsT=S[:],
                                         rhs=msg[:, 0:zc + 4],
                                         start=(t == 0), stop=(t == T - 1))
                    else:
                        nc.tensor.matmul(out=un_ps[:, 0:zc], lhsT=S[:],
                                         rhs=msg[:, 0:zc],
                                         start=(t == 0), stop=(t == T - 1))
                        nc.tensor.matmul(out=den_ps[:, 0:4], lhsT=S[:],
                                         rhs=msg[:, zc:zc + 4],
                                         start=(t == 0), stop=(t == T - 1))

                # ---- chunk epilogue ----
                den32 = sm.tile([128, 4], f32, name="den32", tag="den32")
                if last:
                    nc.vector.tensor_scalar_add(den32[:], un_ps[:, zc:zc + 4], 1e-30)
                else:
                    nc.vector.tensor_scalar_add(den32[:], den_ps[:, 0:4], 1e-30)
                recip = sm.tile([128, 4], f32, name="recip", tag="recip")
                nc.vector.reciprocal(recip[:], den32[:])
                if last:
                    nc.vector.tensor_scalar_mul(recip[:], recip[:], 0.25)
                D = zc // H
                outn = sm.tile([128, 512], f32, name="outn", tag="outn")
                nc.vector.tensor_tensor(
                    out=outn[:, 0:zc].rearrange("p (h d) -> p h d", h=H),
                    in0=un_ps[:, 0:zc].rearrange("p (h d) -> p h d", h=H),
                    in1=recip[:].to_broadcast([128, H, D]),
                    op=OP.mult)
                if last:
                    lg = logits32[:, c * 64:(c + 1) * 64]
                    nc.vector.tensor_add(outn[:, 0:64], outn[:, 0:64], outn[:, 64:128])
                    nc.vector.tensor_add(outn[:, 128:192], outn[:, 128:192],
                                         outn[:, 192:256])
                    nc.vector.tensor_add(outn[:, 0:64], outn[:, 0:64], outn[:, 128:192])
                    nc.vector.tensor_add(lg[:], outn[:, 0:64], bias2[:])
                    nc.sync.dma_start(logits_d[c * CAP:(c + 1) * CAP, :], lg[:])
                    l16 = sm.tile([128, 64], f16, name="l16", tag="l16")
                    nc.vector.tensor_copy(l16[:], lg[:])
                    ltr = ps.tile([128, 128], f16, name="ltr", tag="tr")
                    nc.tensor.transpose(out=ltr[0:64, :], in_=l16[:], identity=id16[:])
                    nc.scalar.activation(ltT[:, c * 128:(c + 1) * 128], ltr[0:64, :],
                                         AF.Copy)
                else:
                    badd = sm.tile([128, 512], f32, name="badd", tag="badd")
                    nc.vector.tensor_tensor(out=badd[:, 0:zc], in0=outn[:, 0:zc],
                                            in1=bias01[:, l * 512:l * 512 + zc],
                                            op=OP.add)
                    nc.scalar.activation(h_next[:, c * zc:(c + 1) * zc],
                                         badd[:, 0:zc], AF.Relu)
            if not last:
                h_cur = h_next

          # ---- decoder ----
          if stage >= 7:
            lt_shard = dram.tile([64, cpc * 128], f16, name=f"lt_shard_{rep}")
            lt_all = dram.tile([NCORES * 64, cpc * 128], f16, addr_space="Shared",
                               name=f"lt_all_{rep}")
            nc.sync.dma_start(lt_shard[:], ltT[:])
            nc.gpsimd.collective_compute(
                "AllGather", mybir.AluOpType.bypass,
                replica_groups=[list(range(NCORES))],
                ins=[lt_shard[:]], outs=[lt_all[:]])
            ltfull = const.tile([64, nrows], f16)
            for g in range(NCORES):
                nc.sync.dma_start(ltfull[0:64, g * cpc * 128:(g + 1) * cpc * 128],
                                  lt_all[g * 64:(g + 1) * 64, :])

            nblocks = nrows // 512
            for c in range(cpc):
                for nb2 in range(nblocks // 2):
                    adj_sb = wk.tile([128, 1024], f32, name="adj_sb", tag="adj")
                    for half in range(2):
                        nb = nb2 * 2 + half
                        adj_ps = ps.tile([128, 512], f32, name="adj_ps", tag="big")
                        nc.tensor.matmul(out=adj_ps[:],
                                         lhsT=ltT[:, c * 128:(c + 1) * 128],
                                         rhs=ltfull[:, nb * 512:(nb + 1) * 512],
                                         start=True, stop=True)
                        if half:
                            nc.scalar.activation(adj_sb[:, half * 512:(half + 1) * 512],
                                                 adj_ps[:], AF.Copy)
                        else:
                            nc.vector.tensor_copy(adj_sb[:, half * 512:(half + 1) * 512],
                                                  adj_ps[:])
                    nc.sync.dma_start(
                        adj_d[c * CAP:(c + 1) * CAP, nb2 * 1024:(nb2 + 1) * 1024],
                        adj_sb[:])

    nc.compile()
    return nc


# ----------------------------------------------------------------------------
# entry point
# ----------------------------------------------------------------------------
def kernel(**inputs):
    from concourse.bass_utils import run_bass_kernel_spmd

    n_nodes = np.asarray(inputs["inputs"]).shape[0]
    cpc = max(1, (n_nodes + NCORES * CAP - 1) // (NCORES * CAP))
    in_maps, perm, T_list = _prep_host(inputs, cpc)

    key = (tuple(T_list), cpc, os.environ.get("KSTAGE", "9"),
           os.environ.get("KREPS", "1"))
    if key not in _PROG_CACHE:
        _PROG_CACHE[key] = _build_program(T_list, cpc)
    nc = _PROG_CACHE[key]

    trace = bool(int(os.environ.get("KERNEL_TRACE", "0")))
    res = run_bass_kernel_spmd(nc, in_maps, core_ids=list(range(NCORES)),
                               trace=trace)
    if trace:
        kernel.last_exec_time_ns = res.exec_time_ns
        kernel.last_results = res

    nrows = NCORES * cpc * CAP
    valid = perm >= 0
    cols = perm[valid]                       # table col -> node, for valid cols
    adj = np.zeros((n_nodes, n_nodes), np.float32)
    logits = np.zeros((n_nodes, 64), np.float32)
    for c in range(NCORES):
        r = res.results[c]
        pv = perm[c * cpc * CAP:(c + 1) * cpc * CAP]
        rv = pv >= 0
        logits[pv[rv]] = r["logits"][rv]
        adj[np.ix_(pv[rv], cols)] = r["adj"][np.ix_(rv, valid)]
    return adj, logits
